# revision 93
# baseline (speedup 1.0000x reference)
"""Trainium2 Bass kernel: Tacotron2-style location-sensitive attention.

Reference computation (per batch b):
    pl[h]    = sum_d lstm[b,d] * W_lstm[h,d]                      # [128]
    loc[c,t] = sum_k conv_w[c,k] * awc_pad[b, t+k]                # same-pad conv
    pa[t,h]  = sum_c loc[c,t] * W_loc[h,c]
             = sum_k G[h,k] * awc_pad[b, t+k],  G = W_loc @ conv_w  # fused
    e[t]     = sum_h W_e[h] * tanh(pl[h] + pa[t,h] + peo[b,t,h])
    att[t]   = softmax_t(e)     (computed without max-subtraction: |e| <~ 10)
    ctx[e]   = sum_t att[t] * enc[b,t,e]

Sharding: data-parallel over batch B=64 across 8 cores (8 batches each);
weights replicated.

Layout strategy (all per-batch, fully pipelined -- no cross-batch barrier):
  * [H=128 part, T free] for the pre-tanh sum: pa via a fused [31,128]
    "G^T" matmul over sliding awc windows, peo added by accumulating PE
    transposes straight into the same PSUM tile, pl added as the
    activation bias of the tanh.
  * energies in [T-part] layout directly: e_T[t] = tanh_tile.T @ W_e^T
    (tanh stationary, N=1), 16 columns -> one [128,16] PSUM tile/batch.
  * exp on ACT with accumulated row sums; total via ones-matmul; 1/sum
    broadcast via a K=1 ones matmul; normalization folded into the final
    context copy and an off-path attw row.
  * context: 16 accumulating [128,1]x[128,512] f32r matmuls per batch.
DMA queues: SP carries only fire-and-forget input streams (awin, peo,
enc, weights); GPSIMD/SWDGE carries compute-dependent output DMAs.
"""

import numpy as np

import concourse.bacc as bacc
import concourse.bass as bass
import concourse.tile as tile
from concourse import mybir
from concourse.bass_utils import run_bass_kernel_spmd
from concourse.masks import make_identity

F32 = mybir.dt.float32
F32R = mybir.dt.float32r
AF = mybir.ActivationFunctionType

NCORES = 8
B, T, E, H, DL, CC, KW = 64, 2048, 512, 128, 1024, 32, 31
BC = B // NCORES          # batches per core
PAD = KW // 2             # 15
SK = 4                    # conv-tap stride factor: k = SK*a + r
NA = 8                    # taps per strided group (SK*NA = 32 >= KW)
AWL = T + SK              # awin2 row length (covers t + r, r < SK)
TP = T + 2 * PAD + 2      # 2080, zero-padded awc length (row 7 spans to 2079)
CH = 4                    # T chunks of 512
CHW = T // CH             # 512
SUB = 128                 # sub-chunk (partition tile) within a chunk
NSUB = CHW // SUB         # 4
NC16 = T // SUB           # 16 sub-chunks per batch


def build_program():
    nc = bacc.Bacc(
        "TRN2",
        target_bir_lowering=False,
        debug=False,
        enable_asserts=True,
        num_devices=NCORES,
    )

    # Layout-prepped inputs (host-side reshape/transpose/pad only -- all
    # arithmetic stays on-chip): awc_pad is zero-padded awc, *_T are
    # pre-transposed copies, ident128 is np.eye.
    enc = nc.declare_dram_parameter("enc", [BC, T, E], F32, isOutput=False).ap()
    peo = nc.declare_dram_parameter("peo", [BC, T, H], F32, isOutput=False).ap()
    lstm_t = nc.declare_dram_parameter("lstm_T", [DL, BC], F32, isOutput=False).ap()
    awc_pad = nc.declare_dram_parameter("awc_pad", [BC, TP], F32,
                                        isOutput=False).ap()
    wl_t = nc.declare_dram_parameter("wl_T", [DL, H], F32, isOutput=False).ap()
    # conv taps regrouped on host (pure reshape): cw4[c, r, a] = cw[c, SK*a+r]
    conv_w4 = nc.declare_dram_parameter("conv_w4", [CC, SK, NA], F32,
                                        isOutput=False).ap()
    wloc_t = nc.declare_dram_parameter("wloc_T", [CC, H], F32, isOutput=False).ap()
    we_t = nc.declare_dram_parameter("we_T", [H, 1], F32, isOutput=False).ap()
    ctx_out = nc.declare_dram_parameter("ctx", [BC, E], F32, isOutput=True).ap()
    attw_out = nc.declare_dram_parameter("attw", [BC, T], F32, isOutput=True).ap()

    from contextlib import ExitStack

    with tile.TileContext(nc) as tc, ExitStack() as es:
        singles = es.enter_context(tc.tile_pool(name="singles", bufs=1))

        # main-pipeline pools (opened early so batch 0's input streams can be
        # emitted at the very head of the SP FIFO, ahead of the setup DMAs)
        awc_pool = es.enter_context(tc.tile_pool(name="awcw", bufs=2))
        peo_pool = es.enter_context(tc.tile_pool(name="peo", bufs=5))

        def emit_awin(b):
            # stride-SK sliding windows: row a holds awc_pad[b, SK*a : SK*a+AWL];
            # the remaining r-shift is applied as a free-dim offset at matmul
            # time, so only NA=8 rows are needed instead of KW=31.
            awin = awc_pool.tile([NA, AWL], F32R, tag="awin")
            win_src = bass.AP(
                tensor=awc_pad.tensor,
                offset=awc_pad.offset + b * TP,
                ap=[[SK, NA], [1, AWL]],
            )
            nc.sync.dma_start(out=awin, in_=win_src.bitcast(F32R))
            return awin

        enc_pool = es.enter_context(tc.tile_pool(name="enc", bufs=8))

        def emit_peo_chunk(b, c):
            tsl = slice(c * CHW, (c + 1) * CHW)
            peo_t = peo_pool.tile([SUB, NSUB, H], F32, tag="peo_t")
            nc.sync.dma_start(
                out=peo_t,
                in_=peo[b, tsl, :].rearrange("(n p) h -> p n h", p=SUB),
            )
            return peo_t

        def emit_enc_chunk(b, c, split=False):
            tsl = slice(c * CHW, (c + 1) * CHW)
            enc_t = enc_pool.tile([SUB, NSUB, E], F32R, tag="enc_t")
            src = enc[b, tsl, :].rearrange("(n p) e -> p n e", p=SUB)
            if split:
                # split the very last transfer so the final context matmuls
                # trail a 256KB sub-transfer, not a 1MB tile
                for j in range(NSUB):
                    nc.sync.dma_start(
                        out=enc_t[:, j, :], in_=src[:, j, :].bitcast(F32R))
            else:
                nc.sync.dma_start(out=enc_t, in_=src.bitcast(F32R))
            return enc_t

        def emit_streams(b):
            awin = emit_awin(b)
            peo_ts = [emit_peo_chunk(b, c) for c in range(CH)]
            enc_ts = [emit_enc_chunk(b, c, split=(b == BC - 1 and c == CH - 1))
                      for c in range(CH)]
            return awin, peo_ts, enc_ts

        # hoist awin+peo of batch 0 (enc would push the setup DMAs, incl. the
        # identity needed by the first transposes, ~14us out). b0's peo goes
        # as ONE 1MB DMA: at startup the SP sequencer's ~1us-per-DMA issue
        # rate can't keep the engines fed with 256KB transfers, so give them
        # a single large backlog instead.
        b0_peo_all = peo_pool.tile([SUB, CH * NSUB, H], F32, tag="peo_b0")
        nc.sync.dma_start(
            out=b0_peo_all,
            in_=peo[0, :, :].rearrange("(n p) h -> p n h", p=SUB),
        )
        b0_peo_ts = [b0_peo_all[:, c * NSUB:(c + 1) * NSUB, :]
                     for c in range(CH)]
        b0_awin = emit_awin(0)

        # identity built on the idle GPSIMD engine -- saves the 64KB input
        # DMA and an SP issue slot at the head of the FIFO
        ident = singles.tile([128, 128], F32)
        make_identity(nc, ident)
        ones_row = singles.tile([1, 128], F32)
        nc.vector.memset(ones_row, 1.0)
        ones_col = singles.tile([128, 1], F32)
        nc.vector.memset(ones_col, 1.0)

        # ---- load weights (split across SP/ACT queues to overlap the
        # per-DMA fixed issue cost at startup; PE setup depends on them) -----
        wlocT_sb = singles.tile([CC, H], F32)
        nc.sync.dma_start(out=wlocT_sb, in_=wloc_t)
        cw_sb = singles.tile([CC, SK, NA], F32)
        nc.scalar.dma_start(out=cw_sb, in_=conv_w4)
        wlT_sb = singles.tile([128, DL // 128, H], F32)
        nc.scalar.dma_start(
            out=wlT_sb, in_=wl_t.rearrange("(n p) h -> p n h", p=128))
        lsT_sb = singles.tile([128, DL // 128, BC], F32)
        nc.scalar.dma_start(
            out=lsT_sb, in_=lstm_t.rearrange("(n p) b -> p n b", p=128))
        weT_sb = singles.tile([H, 1], F32)
        nc.scalar.dma_start(out=weT_sb, in_=we_t)

        with tc.tile_pool(name="ps_setup", bufs=2, space="PSUM") as ps_setup:
            # G4[a, r, h] = sum_c conv_w[c, SK*a+r] * W_loc[h, c]  (full fp32)
            g4_sb = singles.tile([NA, SK, H], F32R)
            for r in range(SK):
                g4_ps = ps_setup.tile([NA, H], F32, tag="wg")
                nc.tensor.matmul(g4_ps, lhsT=cw_sb[:, r, :], rhs=wlocT_sb,
                                 start=True, stop=True)
                nc.vector.tensor_copy(g4_sb[:, r, :], g4_ps)

            # pl^T[h, b] = sum_d W_lstm[h, d] * lstm[b, d]  (full fp32)
            nkc = DL // 128
            plT_ps = ps_setup.tile([H, BC], F32, tag="plT")
            for kc in range(nkc):
                nc.tensor.matmul(
                    plT_ps, lhsT=wlT_sb[:, kc, :], rhs=lsT_sb[:, kc, :],
                    start=(kc == 0), stop=(kc == nkc - 1),
                )
            plT_sb = singles.tile([H, BC], F32)
            nc.vector.tensor_copy(plT_sb, plT_ps)

        # ---- main per-batch pipeline ----------------------------------------
        tanh_pool = es.enter_context(tc.tile_pool(name="tanh", bufs=4))
        att_pool = es.enter_context(tc.tile_pool(name="att", bufs=2))
        out_pool = es.enter_context(tc.tile_pool(name="outs", bufs=2))

        pa_psum = es.enter_context(tc.tile_pool(name="pa_ps", bufs=2, space="PSUM"))
        eT_psum = es.enter_context(tc.tile_pool(name="eT_ps", bufs=2, space="PSUM"))
        ctx_psum = es.enter_context(tc.tile_pool(name="ctx_ps", bufs=2, space="PSUM"))
        misc_psum = es.enter_context(tc.tile_pool(name="misc_ps", bufs=2, space="PSUM"))

        for b in range(BC):
            # -- input streams for this batch (SP FIFO, fire-and-forget) ------
            if b == 0:
                awin, peo_ts = b0_awin, b0_peo_ts
                enc_ts = [emit_enc_chunk(0, c) for c in range(CH)]
            else:
                awin, peo_ts, enc_ts = emit_streams(b)

            # -- energies: e_T[128, 16] with T on partitions ------------------
            eT_ps = eT_psum.tile([SUB, NC16], F32)
            for c in range(CH):
                tsl = slice(c * CHW, (c + 1) * CHW)
                pa_ps = pa_psum.tile([H, CHW], F32, tag="pa")
                # peo^T via accumulating PE transposes; the first one carries
                # start=True (zeroes the whole PSUM zero region), so the pa
                # group can begin before gT is ready
                for j in range(NSUB):
                    nc.tensor.matmul(
                        pa_ps[:, j * SUB:(j + 1) * SUB],
                        lhsT=peo_ts[c][:, j, :],
                        rhs=ident,
                        is_transpose=True,
                        start=(j == 0),
                        stop=False,
                    )
                # pa += sum_r G4_r.T @ (stride-SK awc windows shifted by r)
                # (f32r, N=512 -> full PE rate)
                for r in range(SK):
                    nc.tensor.matmul(
                        pa_ps,
                        lhsT=g4_sb[:, r, :],
                        rhs=awin[:, c * CHW + r:c * CHW + r + CHW],
                        start=False,
                        stop=(r == SK - 1),
                    )
                tanh_sb = tanh_pool.tile([H, CHW], F32, tag="tanh")
                nc.scalar.activation(
                    tanh_sb, pa_ps, func=AF.Tanh,
                    bias=plT_sb[:, b:b + 1], scale=1.0,
                )
                # e_T column per 128-sub-chunk: tanh^T @ W_e^T  (N=1)
                for j in range(NSUB):
                    c16 = c * NSUB + j
                    nc.tensor.matmul(
                        eT_ps[:, c16:c16 + 1],
                        lhsT=tanh_sb[:, j * SUB:(j + 1) * SUB],
                        rhs=weT_sb,
                        start=True, stop=True,
                    )

            # -- per-batch softmax pieces ------------------------------------
            att_expT = att_pool.tile([SUB, NC16], F32R, tag="att_expT")
            eT_sum = att_pool.tile([SUB, 1], F32, tag="eT_sum")
            nc.scalar.activation(
                att_expT, eT_ps, func=AF.Exp, scale=1.0, accum_out=eT_sum,
            )
            tot_ps = misc_psum.tile([1, 1], F32, tag="misc")
            nc.tensor.matmul(tot_ps, lhsT=eT_sum, rhs=ones_col,
                             start=True, stop=True)
            rs_b = att_pool.tile([1, 1], F32, tag="rs_b")
            nc.vector.reciprocal(rs_b, tot_ps)

            # -- normalized attention weights output (does not need enc) -----
            rsb_ps = misc_psum.tile([SUB, 1], F32, tag="misc")
            nc.tensor.matmul(rsb_ps, lhsT=ones_row, rhs=rs_b,
                             start=True, stop=True)
            rs_bc = att_pool.tile([SUB, 1], F32, tag="rs_bc")
            nc.vector.tensor_copy(rs_bc, rsb_ps)
            attwT_sb = att_pool.tile([SUB, NC16], F32, tag="attwT")
            nc.vector.tensor_scalar_mul(attwT_sb, att_expT, rs_bc)
            attw_ps = misc_psum.tile([NC16, SUB], F32, tag="misc")
            nc.tensor.matmul(attw_ps, lhsT=attwT_sb, rhs=ident,
                             is_transpose=True, start=True, stop=True)
            attw_row = out_pool.tile([NC16, SUB], F32, tag="attw_row")
            nc.vector.tensor_copy(attw_row, attw_ps)
            # compute-dependent outs go to SWDGE (own sem-lane space, POOL
            # idle); last batch uses ACT's HWDGE -- faster issue, and there
            # are no later input DMAs to head-of-line block.
            out_eng = nc.scalar if b == BC - 1 else nc.gpsimd
            out_eng.dma_start(
                out=attw_out[b, :].rearrange("(c j) -> c j", c=NC16),
                in_=attw_row,
            )

            # -- context ------------------------------------------------------
            ctx_ps = ctx_psum.tile([1, E], F32, tag="ctx")
            for c in range(CH):
                for j in range(NSUB):
                    c16 = c * NSUB + j
                    nc.tensor.matmul(
                        ctx_ps,
                        lhsT=att_expT[:, c16:c16 + 1],
                        rhs=enc_ts[c][:, j, :],
                        start=(c16 == 0),
                        stop=(c16 == NC16 - 1),
                    )
            ctx_sb = out_pool.tile([1, E], F32, tag="ctx_sb")
            nc.scalar.activation(ctx_sb, ctx_ps, func=AF.Copy, scale=rs_b)
            out_eng.dma_start(out=ctx_out[b:b + 1, :], in_=ctx_sb)

    nc.compile()
    return nc


_NC_CACHE = None
LAST_RESULTS = None


def _get_program():
    global _NC_CACHE
    if _NC_CACHE is None:
        _NC_CACHE = build_program()
    return _NC_CACHE


def prep_in_maps(encoder_output, processed_encoder_output, lstm_output,
                 attention_weights_cum, W_lstm, conv_w, W_loc, W_e):
    """Host-side layout prep (pad/transpose/reshape only) + batch sharding."""
    enc = np.ascontiguousarray(encoder_output, dtype=np.float32)
    peo = np.ascontiguousarray(processed_encoder_output, dtype=np.float32)
    lstm = np.asarray(lstm_output, dtype=np.float32).reshape(-1, DL)
    awc = np.asarray(attention_weights_cum, dtype=np.float32)
    nb = enc.shape[0]

    awc_pad = np.zeros((nb, TP), np.float32)
    awc_pad[:, PAD:PAD + T] = awc
    wl_t = np.ascontiguousarray(np.asarray(W_lstm, np.float32).T)      # [DL,H]
    # regroup conv taps k = SK*a + r -> cw4[c, r, a] (zero-padded to SK*NA)
    cw = np.asarray(conv_w, np.float32).reshape(CC, KW)
    cw_pad = np.zeros((CC, SK * NA), np.float32)
    cw_pad[:, :KW] = cw
    cw4 = np.ascontiguousarray(
        cw_pad.reshape(CC, NA, SK).transpose(0, 2, 1))                 # [CC,SK,NA]
    wloc_t = np.ascontiguousarray(np.asarray(W_loc, np.float32).T)     # [CC,H]
    we_t = np.ascontiguousarray(np.asarray(W_e, np.float32).T)         # [H,1]

    in_maps = []
    for i in range(nb // BC):
        sl = slice(i * BC, (i + 1) * BC)
        in_maps.append({
            "enc": np.ascontiguousarray(enc[sl]),
            "peo": np.ascontiguousarray(peo[sl]),
            "lstm_T": np.ascontiguousarray(lstm[sl].T),
            "awc_pad": np.ascontiguousarray(awc_pad[sl]),
            "wl_T": wl_t,
            "conv_w4": cw4,
            "wloc_T": wloc_t,
            "we_T": we_t,
        })
    return in_maps


def kernel(encoder_output, processed_encoder_output, lstm_output,
           attention_weights_cum, W_lstm, conv_w, W_loc, W_e):
    global LAST_RESULTS
    nc = _get_program()
    in_maps = prep_in_maps(
        encoder_output, processed_encoder_output, lstm_output,
        attention_weights_cum, W_lstm, conv_w, W_loc, W_e)

    # transient device wedges (e.g. NRT_EXEC_UNIT_UNRECOVERABLE right after
    # a heavy prior session) have been observed to clear on plain retry
    last_err = None
    for attempt in range(3):
        try:
            res = run_bass_kernel_spmd(nc, in_maps, list(range(NCORES)))
            break
        except Exception as e:  # noqa: BLE001 - retry transient runtime errors
            last_err = e
            msg = str(e)
            if ("UNAVAILABLE" not in msg and "UNRECOVERABLE" not in msg
                    and "NRT" not in msg):
                raise
            import time
            time.sleep(2.0 * (attempt + 1))
    else:
        raise last_err
    LAST_RESULTS = res

    ctx = np.concatenate([res.results[i]["ctx"] for i in range(NCORES)], axis=0)
    attw = np.concatenate([res.results[i]["attw"] for i in range(NCORES)], axis=0)
    return ctx[:, None, :].astype(np.float32), attw.astype(np.float32)
